# revision 89
# baseline (speedup 1.0000x reference)
"""Self-attention kernel for TRN2: out = softmax(X Wq (X Wk)^T / sqrt(D)) @ X.

Strategy (8-way sequence parallelism over query rows):
  scores = (X Wqs)(X Wk)^T = X A  with  A^T = Wk (Wqs^T X_i^T), Wqs = Wq/sqrt(D)
so K and M = Wqs Wk^T are never materialized. Each core i handles query rows
[i*B, (i+1)*B):
  phase 0 (two streamed GEMM steps, per B-half):
    step1: T1 = Wqs^T X_i^T  (3-pass f32r hi/lo: hh+hl+lh — T1 errors amplify
           by ~D into the logits, so crosses must stay near-exact)
    step2: A^T = Wk T1       (3-pass f32r; same sensitivity via sqrt(D)*|X|)
    outputs: A^T as f32r hi tiles (aith) + e4m3 cross operands
           h8a = hi(A)*2^-9, l8a = lo(A)*2^4 in DoubleRow pair layout.
  flash:   stream key blocks j; logits S^T_j = X_j A in key-major layout as
           f32r hh pass + TWO fp8 DoubleRow cross passes (each contracts 256
           per instruction = half the instructions of an f32r pass):
             term1 = lo(X)*2^9 (stationary) x hi(A)*2^-9 (moving)
             term2 = hi(X)*2^-4 x lo(A)*2^4
           running column-max via PE transpose + reduce; E = exp(S - max)
           in bf16; out matmul E^T-slices @ X (bf16); the softmax
           denominator via ones-stationary matmuls into a [1,B] row, with
           the same rescale chain (crow = corr transposed); fused
           rescale-accumulate (acc = acc*corr + psum) on DVE; final divide.

All hi/lo splits are host-precomputed: hi carries an 11-bit mantissa so the
PE's FP22 truncation and the on-device f32r casts are numeric no-ops, and
lo = x - hi is exact; X's fp8 cross operands ship pre-quantized.

Numerics: logits need ~17-bit abs precision (std ~1024, near-tie rows
amplify errors through softmax). f32r hi/lo split leaves cross terms at
~2^-12 relative, so quantizing THOSE to e4m3 (3-bit mantissa) only adds
~7e-3 logit noise — well under the ~0.04 budget (measured rel err 4.6e-3
vs the 2e-2 gate). The P@X matmul only needs ~1e-3 relative, so bf16 with
denominator cancellation is safe there. Phase-0 intermediates amplify by
~D into the logits, so their cross passes must stay f32r.

Software pipeline: PE queue per super-block is [S(s) | out(s-1) |
transposes(s)] with the max-broadcast/exp/xar chains on DVE/ScalarE/GpSimd
hidden under the bursts; PE measures ~93% busy at full clock (idle gaps
also drop the PE p-state 2.4->1.2 GHz, so gaps cost double).
"""
import numpy as np
from contextlib import ExitStack

import concourse.bass as bass
import concourse.bacc as bacc
import concourse.tile as tile
from concourse import mybir
from concourse.bass_utils import run_bass_kernel_spmd
from concourse.masks import make_identity

P = 128
SEQ = 8192
DIM = 1024
NCORES = 8
AUG = 0      # denominator handled by ones-stationary matmuls, not aug cols
SBN = 4      # key n-tiles (of 128) per flash super-block

F32 = mybir.dt.float32
F32R = mybir.dt.float32r
BF16 = mybir.dt.bfloat16
F8 = mybir.dt.float8e4
EXP = mybir.ActivationFunctionType.Exp
ALU = mybir.AluOpType
AXX = mybir.AxisListType.X
DR = mybir.MatmulPerfMode.DoubleRow

# fp8 cross-term scales (product of each pair = 1.0)
S_LX = 512.0      # lo(X) * 2^9   (stationary, term1)
S_HA = 1.0 / 512.0  # hi(A) * 2^-9  (moving, term1)
S_HX = 1.0 / 16.0   # hi(X) * 2^-4  (stationary, term2)
S_LA = 16.0         # lo(A) * 2^4   (moving, term2)


def _chunks(total, step=512):
    return [(lo, min(lo + step, total)) for lo in range(0, total, step)]


def build_core_kernel(S, D, B, sbn=SBN, aug=AUG):
    """One core's kernel: query rows block of size B, full S keys."""
    KT = D // P      # contraction tiles over D
    NT = S // P      # key tiles
    MT = B // P      # query tiles (per core)
    NSB = NT // sbn  # super-blocks
    NPAIR = KT // 2  # DoubleRow contraction pairs
    XAW = D + aug
    assert NT % sbn == 0 and B % P == 0 and D % P == 0 and MT <= P and KT % 2 == 0

    nc = bacc.Bacc("TRN2", target_bir_lowering=False, debug=False)
    # All hi/lo f32r splits are precomputed on the host: hi values carry an
    # 11-bit mantissa (so the on-device f32r cast that satisfies the BIR
    # verifier is numerically a no-op), lo = x - hi exactly. The fp8 cross
    # operands for X ship pre-quantized in the natural (plain DoubleRow)
    # stationary layout.
    xtj = nc.dram_tensor("xtj", [NT, P, D], F32, kind="ExternalInput")
    xl8 = nc.dram_tensor("xl8", [NT, P, D], F8, kind="ExternalInput")
    xh8 = nc.dram_tensor("xh8", [NT, P, D], F8, kind="ExternalInput")
    xa = nc.dram_tensor("xa", [S, XAW], BF16, kind="ExternalInput")
    wqh = nc.dram_tensor("wqh", [KT, P, D], F32, kind="ExternalInput")
    wql = nc.dram_tensor("wql", [KT, P, D], F32, kind="ExternalInput")
    wkh = nc.dram_tensor("wkh", [KT, P, D], F32, kind="ExternalInput")
    wkl = nc.dram_tensor("wkl", [KT, P, D], F32, kind="ExternalInput")
    xih = nc.dram_tensor("xih", [KT, P, B], F32, kind="ExternalInput")
    xil = nc.dram_tensor("xil", [KT, P, B], F32, kind="ExternalInput")
    out = nc.dram_tensor("out", [B, D], F32, kind="ExternalOutput")
    dscr = nc.dram_tensor("dscr", [1, B], F32, kind="Internal")

    def pair_st(t, u):
        # stationary fp8 pair view [P, 2, P] of a [P, D] tile, pair u
        return t[:, u * 2 * P:(u + 1) * 2 * P].rearrange("p (i m) -> p i m", i=2)

    with tile.TileContext(nc) as tc, ExitStack() as ctx:
        pers = ctx.enter_context(tc.tile_pool(name="pers", bufs=1))
        aith = [pers.tile([P, B], F32R, name=f"aith{k}") for k in range(KT)]
        h8a = pers.tile([P, KT, B], F8, name="h8a")
        l8a = pers.tile([P, KT, B], F8, name="l8a")
        gm = pers.tile([P, B], F32, name="gm")
        mxbc = pers.tile([P, B], F32, name="mxbc")
        ident = pers.tile([P, P], F32, name="ident")
        ones = pers.tile([P, P], BF16, name="ones")
        den = pers.tile([1, B], F32, name="den")
        make_identity(nc, ident[:])
        nc.gpsimd.memset(gm[:], -1e30)
        nc.gpsimd.memset(ones[:], 1.0)
        nc.gpsimd.memset(den[:], 0.0)

        # ---- phase 0: T1 = Wqs^T X_i^T ; A^T = Wk T1 (per B-half) ----
        with ExitStack() as p0:
            t1p = p0.enter_context(tc.tile_pool(name="t1p", bufs=1))
            wp = p0.enter_context(tc.tile_pool(name="wp", bufs=2))
            xip = p0.enter_context(tc.tile_pool(name="xip", bufs=1))
            auxp = p0.enter_context(tc.tile_pool(name="auxp", bufs=2))
            ps0 = p0.enter_context(tc.tile_pool(name="ps0", bufs=4, space="PSUM"))
            HB = 512
            hT1 = t1p.tile([P, KT, HB], F32R, name="hT1")
            lT1 = t1p.tile([P, KT, HB], F32R, name="lT1")
            def load_w(dh, dl, col, tag):
                # one strided DMA per hi/lo + f32r casts (the rounding casts
                # keep the BIR verifier happy; numerically no-ops)
                wf_h = wp.tile([P, KT, P], F32, name=f"wfh{tag}", tag="wqfh")
                wf_l = wp.tile([P, KT, P], F32, name=f"wfl{tag}", tag="wqfl")
                nc.sync.dma_start(wf_h[:], dh.ap()[:, :, col * P:(col + 1) * P].rearrange("g p c -> p g c"))
                nc.sync.dma_start(wf_l[:], dl.ap()[:, :, col * P:(col + 1) * P].rearrange("g p c -> p g c"))
                hw = wp.tile([P, KT, P], F32R, name=f"hw{tag}", tag="hwq")
                nc.scalar.copy(hw[:], wf_h[:])
                lw = wp.tile([P, KT, P], F32R, name=f"lw{tag}", tag="lwq")
                nc.vector.tensor_copy(lw[:], wf_l[:])
                return hw, lw

            for (lo, hi) in _chunks(B):
                # wq r=0 first: its DMA would otherwise queue behind all 16
                # X-split DMAs and delay the first matmul by ~10us
                wq0 = load_w(wqh, wql, 0, f"q{lo}_0")

                # X_i^T half: host-split hi/lo, device just casts to f32r
                # (numeric no-op; satisfies the verifier's rounding rule),
                # pipelined per g-tile so step1 starts as soon as g=0 lands
                xif_h = xip.tile([P, KT, HB], F32, name=f"xifh{lo}", tag="xifh")
                xif_l = xip.tile([P, KT, HB], F32, name=f"xifl{lo}", tag="xifl")
                hxi = xip.tile([P, KT, HB], F32R, name=f"hxi{lo}", tag="hxi")
                lxi = xip.tile([P, KT, HB], F32R, name=f"lxi{lo}", tag="lxi")
                for g in range(KT):
                    nc.sync.dma_start(xif_h[:, g, :], xih.ap()[g, :, lo:hi])
                    nc.scalar.copy(hxi[:, g, :], xif_h[:, g, :])
                    nc.sync.dma_start(xif_l[:, g, :], xil.ap()[g, :, lo:hi])
                    nc.vector.tensor_copy(lxi[:, g, :], xif_l[:, g, :])

                # step1: T1[r-tile, half] = sum_g Wqs[g,:][:, r]^T X^T[g, half]
                for r in range(KT):
                    hwq, lwq = wq0 if r == 0 else load_w(wqh, wql, r, f"q{lo}_{r}")
                    t1ps = ps0.tile([P, HB], F32, name=f"t1ps{lo}_{r}", tag="pm")
                    n = 3 * KT
                    i = 0
                    for g in range(KT):
                        for (la, rb) in ((hwq, hxi), (hwq, lxi), (lwq, hxi)):
                            nc.tensor.matmul(t1ps[:], la[:, g, :], rb[:, g, :],
                                             start=(i == 0), stop=(i == n - 1))
                            i += 1
                    nc.vector.tensor_copy(hT1[:, r, :], t1ps[:])
                    nc.vector.tensor_sub(t1ps[:], t1ps[:], hT1[:, r, :].bitcast(F32))
                    nc.vector.tensor_copy(lT1[:, r, :], t1ps[:])

                # step2: A^T[d-tile, half] = sum_r Wk[:, r][d, :] T1[r, half]
                # (reuses the step1 weight-split slots: same tags/shapes)
                for d in range(KT):
                    hwk, lwk = load_w(wkh, wkl, d, f"k{lo}_{d}")
                    pa = ps0.tile([P, HB], F32, name=f"pa{lo}_{d}", tag="pm")
                    n = 3 * KT
                    i = 0
                    for r in range(KT):
                        for (la, rb) in ((hwk, hT1), (hwk, lT1), (lwk, hT1)):
                            nc.tensor.matmul(pa[:], la[:, r, :], rb[:, r, :],
                                             start=(i == 0), stop=(i == n - 1))
                            i += 1
                    nc.vector.tensor_copy(aith[d][:, lo:hi], pa[:])
                    nc.scalar.mul(h8a[:, d, lo:hi], aith[d][:, lo:hi].bitcast(F32), S_HA)
                    al_f = auxp.tile([P, HB], F32, name=f"alf{lo}_{d}", tag="alf")
                    nc.vector.tensor_sub(al_f[:], pa[:], aith[d][:, lo:hi].bitcast(F32))
                    nc.vector.tensor_scalar_mul(l8a[:, d, lo:hi], al_f[:], S_LA)

        # ---- flash over key super-blocks ----
        accp = ctx.enter_context(tc.tile_pool(name="accp", bufs=1))
        acc = [accp.tile([P, XAW], F32, name=f"acc{t}") for t in range(MT)]
        for t in range(MT):
            nc.gpsimd.memset(acc[t][:], 0.0)
        sp = ctx.enter_context(tc.tile_pool(name="sp", bufs=2 * sbn + 1))
        erp = ctx.enter_context(tc.tile_pool(name="erp", bufs=2 * sbn))
        xap = ctx.enter_context(tc.tile_pool(name="xap", bufs=3))
        xarp = ctx.enter_context(tc.tile_pool(name="xarp", bufs=2 * sbn))
        xtp = xap
        xthp = ctx.enter_context(tc.tile_pool(name="xthp", bufs=3))
        stat = ctx.enter_context(tc.tile_pool(name="stat", bufs=2))
        ps_s = ctx.enter_context(tc.tile_pool(name="ps_s", bufs=2, space="PSUM"))
        ps_o = ctx.enter_context(tc.tile_pool(name="ps_o", bufs=2, space="PSUM"))
        ps_d = ctx.enter_context(tc.tile_pool(name="ps_d", bufs=1, space="PSUM"))
        ps_t = ps_s

        # Software pipeline, one-super-block lag, tuned so the PE queue is
        # [S(s) | out(s-1) | transposes(s) | S(s+1) | ...] with no waits:
        # out(s-1)'s operands (er/xar/corr) were finished during S(s)'s burst,
        # and the gm column-maxes feeding transposes(s) finish during
        # out(s-1). The exp chain for s runs on DVE/ScalarE under S(s+1).
        # E and X_aug are bf16 for the out matmul (same 1 cyc/row as f32r,
        # half the SBUF; E's 2^-9 rounding cancels through the ones-column
        # denominator, X_aug's is ~2e-3 of |x| — both far under budget).
        # X_aug ships from the host already in bf16, so no on-device cast.
        def prep_block(s):
            js = list(range(s * sbn, (s + 1) * sbn))
            xsplit = []
            for j in js:
                xt_t = xtp.tile([P, D], F32, name=f"xt{j}", tag="stg")
                nc.sync.dma_start(xt_t[:], xtj.ap()[j])
                xth = xthp.tile([P, D], F32R, name=f"xth{j}", tag="xth")
                nc.scalar.copy(xth[:], xt_t[:])
                l8x = xthp.tile([P, D], F8, name=f"l8x{j}", tag="l8x", bufs=2)
                nc.sync.dma_start(l8x[:], xl8.ap()[j])
                h8x = xthp.tile([P, D], F8, name=f"h8x{j}", tag="h8x", bufs=2)
                nc.sync.dma_start(h8x[:], xh8.ap()[j])
                xsplit.append((xth, l8x, h8x))
            return xsplit

        def s_burst(s, xsplit):
            ssb = []
            for idx, j in enumerate(range(s * sbn, (s + 1) * sbn)):
                xth, l8x, h8x = xsplit[idx]
                s_t = sp.tile([P, B], F32, name=f"s{j}", tag="s")
                pss = [ps_s.tile([P, 512], F32, name=f"pss{j}_{c}", tag="pss")
                       for c in range(2)]
                # f32r hh pass, both chunks back-to-back (same PE mode)
                for c, (lo, hi) in enumerate(_chunks(B)):
                    for k in range(KT):
                        nc.tensor.matmul(pss[c][:], xth[:, k * P:(k + 1) * P],
                                         aith[k][:, lo:hi], start=(k == 0), stop=(k == KT - 1))
                # fp8 DoubleRow cross passes: each accumulates onto the
                # closed f32r group via has_written (start=False); stop is
                # sim-only bookkeeping so every DR matmul closes itself.
                # Chunk 0 drains (copy + running max) while chunk 1's fp8
                # matmuls stream, so the stats transposes can start the
                # moment the burst ends.
                for c, (lo, hi) in enumerate(_chunks(B)):
                    for u in range(NPAIR):
                        nc.tensor.matmul(pss[c][:], pair_st(l8x, u),
                                         h8a[:, 2 * u:2 * u + 2, lo:hi],
                                         start=False, stop=True, perf_mode=DR,
                                         skip_group_check=True)
                    for u in range(NPAIR):
                        nc.tensor.matmul(pss[c][:], pair_st(h8x, u),
                                         l8a[:, 2 * u:2 * u + 2, lo:hi],
                                         start=False, stop=True, perf_mode=DR,
                                         skip_group_check=True)
                    nc.scalar.copy(s_t[:, lo:hi], pss[c][:])
                    nc.vector.tensor_max(gm[:, lo:hi], gm[:, lo:hi], pss[c][:])
                ssb.append(s_t)
            return ssb

        def stats_block(s, omx, omrow):
            # per-query-column running max (transpose-reduce gm chunks)
            nmx = stat.tile([P, MT], F32, name=f"nmx{s}", tag="nmx")
            corr = stat.tile([P, MT], F32, name=f"corr{s}", tag="corr")
            for c in range(MT):
                pt = ps_t.tile([P, P], F32, name=f"pt{s}_{c}", tag="pss")
                nc.tensor.transpose(pt[:], gm[:, c * P:(c + 1) * P], ident[:])
                nc.vector.reduce_max(nmx[:, c:c + 1], pt[:], axis=AXX)
            if omx is None:
                nc.vector.memset(corr[:], 0.0)
            else:
                dmx = stat.tile([P, MT], F32, name=f"dmx{s}", tag="dmx")
                nc.vector.tensor_sub(dmx[:], omx[:], nmx[:])
                nc.scalar.activation(corr[:], dmx[:], EXP)

            # broadcast nmx (query-major) -> mxbc [P, B] (key-major free)
            ptb = ps_t.tile([P, P], F32, name=f"ptb{s}", tag="pss")
            nc.tensor.transpose(ptb[:MT, :], nmx[:], ident[:])
            mtmp = stat.tile([MT, P], F32, name=f"mtmp{s}", tag="mtmp")
            nc.scalar.copy(mtmp[:], ptb[:MT, :])
            # issue the tiny mrow DMA from the scalar queue so it doesn't
            # sit behind the bulk xtj/xa loads on the sync queue
            mrow = stat.tile([1, B], F32, name=f"mrow{s}", tag="mrow", bufs=2)
            nc.scalar.dma_start(mrow[:].rearrange("a (b c) -> a b c", b=MT), mtmp[:])
            nc.gpsimd.partition_broadcast(mxbc[:], mrow[:])
            # crow = exp(omrow - mrow) == corr transposed to row space, but
            # computed directly from the max rows: saves a PE transpose +
            # copy + DMA per super-block in the denominator chain
            crow = stat.tile([1, B], F32, name=f"crow{s}", tag="crow", bufs=2)
            if omrow is None:
                nc.vector.memset(crow[:], 0.0)
            else:
                nc.vector.tensor_sub(crow[:], omrow[:], mrow[:])
                nc.scalar.activation(crow[:], crow[:], EXP)
            return nmx, corr, mrow, crow

        def exp_block(s, ssb):
            # E = exp(S - max), exp writes bf16 er (out dtype converts).
            # Chunked [P, 512] with chunk 0 of every tile first: the out
            # burst consumes er columns t*128.. in t order, so all its
            # early stationaries come from chunk 0 — this halves the time
            # from max-broadcast to out-burst start.
            ers = [erp.tile([P, B], BF16, name=f"er{s}_{idx}", tag="er")
                   for idx in range(len(ssb))]
            for (lo, hi) in _chunks(B):
                for idx, s_t in enumerate(ssb):
                    nc.vector.tensor_sub(s_t[:, lo:hi], s_t[:, lo:hi], mxbc[:, lo:hi])
                    nc.scalar.activation(ers[idx][:, lo:hi], s_t[:, lo:hi], EXP)
            return ers

        def xar_block(s):
            xar = []
            for j in range(s * sbn, (s + 1) * sbn):
                xa_t = xarp.tile([P, XAW], BF16, name=f"xa{j}", tag="xar")
                nc.sync.dma_start(xa_t[:], xa.ap()[j * P:(j + 1) * P, :])
                xar.append(xa_t)
            return xar

        def den_update(s, ers, crow):
            # softmax denominator via ones-stationary matmuls (row layout),
            # rescaled with the row-space crow from stats_block
            dps = ps_d.tile([P, B], F32, name=f"dps{s}", tag="dps")
            for (lo, hi) in _chunks(B):
                for idx in range(sbn):
                    nc.tensor.matmul(dps[:, lo:hi], ones[:], ers[idx][:, lo:hi],
                                     start=(idx == 0), stop=(idx == sbn - 1))
            nc.vector.tensor_mul(den[:], den[:], crow[:])
            nc.vector.tensor_add(den[:], den[:], dps[0:1, :])

        def out_block(s, ers, xar, corr, crow, final=False):
            # out accumulation: acc = acc*corr + E^T @ X (bf16 burst).
            # On the final block the denominator runs FIRST so its
            # row->query-major DRAM bounce finishes during the burst and the
            # per-tile divides+stores pipeline with the matmuls.
            rcd = None
            if final:
                den_update(s, ers, crow)
                nc.sync.dma_start(dscr.ap()[:, :], den[:])
                dtmp = stat.tile([P, P], F32, name="dtmp", tag="dtmp")
                nc.gpsimd.memset(dtmp[:], 0.0)
                nc.sync.dma_start(dtmp[:MT, :], dscr.ap()[0, :].rearrange("(b c) -> b c", b=MT))
            for t in range(MT):
                po = ps_o.tile([P, XAW], F32, name=f"po{s}_{t}", tag="po")
                # idx outer so the column chunks reuse one stationary
                # operand back-to-back (LDWEIGHTS locality); each chunk's
                # PSUM accumulation group still spans idx 0..sbn-1
                for idx in range(sbn):
                    er = ers[idx][:]
                    for (lo, hi) in _chunks(XAW):
                        nc.tensor.matmul(po[:, lo:hi], er[:, t * P:(t + 1) * P],
                                         xar[idx][:, lo:hi], start=(idx == 0), stop=(idx == sbn - 1))
                nc.vector.scalar_tensor_tensor(acc[t][:], acc[t][:],
                                               corr[:, t:t + 1], po[:],
                                               op0=ALU.mult, op1=ALU.add)
                if final and t == 2:
                    ptd = ps_t.tile([P, P], F32, name="ptd", tag="pss")
                    nc.tensor.transpose(ptd[:], dtmp[:], ident[:])
                    rcd = stat.tile([P, MT], F32, name="rcd", tag="rcd")
                    nc.vector.reciprocal(rcd[:], ptd[:, :MT])
                if final and t >= 2:
                    for tt in ([0, 1, 2] if t == 2 else [t]):
                        nc.vector.tensor_scalar_mul(acc[tt][:], acc[tt][:], rcd[:, tt:tt + 1])
                        eng = nc.sync if tt % 2 == 0 else nc.scalar
                        eng.dma_start(out.ap()[tt * P:(tt + 1) * P, :], acc[tt][:])
            if not final:
                den_update(s, ers, crow)

        omx = None
        omrow = None
        prev = None     # out_block args for block s-1
        xsplit = prep_block(0)
        for s in range(NSB):
            ssb = s_burst(s, xsplit)
            if s + 1 < NSB:
                xsplit = prep_block(s + 1)
            # stats(s) on PE right after the burst (its gm maxes already
            # drained), so the max-broadcast/exp chain overlaps out(s-1)
            nmx, corr, mrow, crow = stats_block(s, omx, omrow)
            omx = nmx
            omrow = mrow
            if prev is not None:
                out_block(s - 1, *prev)
            ers = exp_block(s, ssb)
            xar = xar_block(s)
            prev = (ers, xar, corr, crow)
        out_block(NSB - 1, *prev, final=True)

    nc.compile()
    return nc


def _split_f32r(x):
    """Host replica of the f32r hi/lo split: hi = x rounded (half-up) to an
    11-bit mantissa — so the PE's FP22 truncation and the device's f32r cast
    both read it back exactly — and lo = x - hi, exact in fp32."""
    x = np.ascontiguousarray(x, np.float32)
    hi = ((x.view(np.uint32) + np.uint32(0x800)) & np.uint32(0xFFFFF000)).view(np.float32)
    return hi, (x - hi).astype(np.float32)


def prep_inputs(X, Wq, Wk, S, D, n_cores, aug=AUG):
    import ml_dtypes
    F8NP = ml_dtypes.float8_e4m3
    B = S // n_cores
    NT = S // P
    KT = D // P
    X = np.ascontiguousarray(X, np.float32)
    scale = np.float32(1.0 / np.sqrt(D))
    xtj = np.ascontiguousarray(
        X.reshape(NT, P, KT, P).transpose(0, 3, 2, 1).reshape(NT, P, D))
    xtj_hi, xtj_lo = _split_f32r(xtj)
    xl8 = (xtj_lo * np.float32(S_LX)).astype(F8NP)
    xh8 = (xtj_hi * np.float32(S_HX)).astype(F8NP)
    xa = np.zeros((S, D + aug), ml_dtypes.bfloat16)
    xa[:, :D] = X.astype(ml_dtypes.bfloat16)
    if aug:
        xa[:, D] = 1.0
    wqh, wql = _split_f32r(np.asarray(Wq, np.float32) * scale)
    wkh, wkl = _split_f32r(np.asarray(Wk, np.float32).T)
    xih_full, xil_full = _split_f32r(X.T)
    in_maps = []
    for i in range(n_cores):
        in_maps.append({
            "xtj": xtj_hi, "xl8": xl8, "xh8": xh8, "xa": xa,
            "wqh": wqh.reshape(KT, P, D), "wql": wql.reshape(KT, P, D),
            "wkh": wkh.reshape(KT, P, D), "wkl": wkl.reshape(KT, P, D),
            "xih": np.ascontiguousarray(xih_full[:, i * B:(i + 1) * B]).reshape(KT, P, B),
            "xil": np.ascontiguousarray(xil_full[:, i * B:(i + 1) * B]).reshape(KT, P, B),
        })
    return in_maps


_CACHE = {}


def _get_kernel(S, D, B, sbn):
    key = (S, D, B, sbn)
    if key not in _CACHE:
        _CACHE[key] = build_core_kernel(S, D, B, sbn=sbn)
    return _CACHE[key]


def kernel(inputs, weight_query, weight_key):
    S, D = inputs.shape
    assert (S, D) == (SEQ, DIM)
    B = S // NCORES
    nc = _get_kernel(S, D, B, SBN)
    in_maps = prep_inputs(inputs, weight_query, weight_key, S, D, NCORES)
    res = run_bass_kernel_spmd(nc, in_maps, core_ids=list(range(NCORES)))
    return np.concatenate([res.results[i]["out"] for i in range(NCORES)], axis=0)


if __name__ == "__main__":
    rng = np.random.default_rng(0)
    X = rng.standard_normal((SEQ, DIM), dtype=np.float32)
    Wq = rng.standard_normal((DIM, DIM), dtype=np.float32)
    Wk = rng.standard_normal((DIM, DIM), dtype=np.float32)
    out = kernel(X, Wq, Wk)
    print(out.shape, out.dtype)


# revision 90
# speedup vs baseline: 1.0064x; 1.0064x over previous
"""Self-attention kernel for TRN2: out = softmax(X Wq (X Wk)^T / sqrt(D)) @ X.

Strategy (8-way sequence parallelism over query rows):
  scores = (X Wqs)(X Wk)^T = X A  with  A^T = Wk (Wqs^T X_i^T), Wqs = Wq/sqrt(D)
so K and M = Wqs Wk^T are never materialized. Each core i handles query rows
[i*B, (i+1)*B):
  phase 0 (two streamed GEMM steps, per B-half):
    step1: T1 = Wqs^T X_i^T  (3-pass f32r hi/lo: hh+hl+lh — T1 errors amplify
           by ~D into the logits, so crosses must stay near-exact)
    step2: A^T = Wk T1       (3-pass f32r; same sensitivity via sqrt(D)*|X|)
    outputs: A^T as f32r hi tiles (aith) + e4m3 cross operands
           h8a = hi(A)*2^-9, l8a = lo(A)*2^4 in DoubleRow pair layout.
  flash:   stream key blocks j; logits S^T_j = X_j A in key-major layout as
           f32r hh pass + TWO fp8 DoubleRow cross passes (each contracts 256
           per instruction = half the instructions of an f32r pass):
             term1 = lo(X)*2^9 (stationary) x hi(A)*2^-9 (moving)
             term2 = hi(X)*2^-4 x lo(A)*2^4
           running column-max via PE transpose + reduce; E = exp(S - max)
           in bf16; out matmul E^T-slices @ X (bf16); the softmax
           denominator via ones-stationary matmuls into a [1,B] row, with
           the same rescale chain (crow = corr transposed); fused
           rescale-accumulate (acc = acc*corr + psum) on DVE; final divide.

All hi/lo splits are host-precomputed: hi carries an 11-bit mantissa so the
PE's FP22 truncation and the on-device f32r casts are numeric no-ops, and
lo = x - hi is exact; X's fp8 cross operands ship pre-quantized.

Numerics: logits need ~17-bit abs precision (std ~1024, near-tie rows
amplify errors through softmax). f32r hi/lo split leaves cross terms at
~2^-12 relative, so quantizing THOSE to e4m3 (3-bit mantissa) only adds
~7e-3 logit noise — well under the ~0.04 budget (measured rel err 4.6e-3
vs the 2e-2 gate). The P@X matmul only needs ~1e-3 relative, so bf16 with
denominator cancellation is safe there. Phase-0 intermediates amplify by
~D into the logits, so their cross passes must stay f32r.

Software pipeline: PE queue per super-block is [S(s) | out(s-1) |
transposes(s)] with the max-broadcast/exp/xar chains on DVE/ScalarE/GpSimd
hidden under the bursts; PE measures ~93% busy at full clock (idle gaps
also drop the PE p-state 2.4->1.2 GHz, so gaps cost double).
"""
import numpy as np
from contextlib import ExitStack

import concourse.bass as bass
import concourse.bacc as bacc
import concourse.tile as tile
from concourse import mybir
from concourse.bass_utils import run_bass_kernel_spmd
from concourse.masks import make_identity

P = 128
SEQ = 8192
DIM = 1024
NCORES = 8
AUG = 0      # denominator handled by ones-stationary matmuls, not aug cols
SBN = 4      # key n-tiles (of 128) per flash super-block

F32 = mybir.dt.float32
F32R = mybir.dt.float32r
BF16 = mybir.dt.bfloat16
F8 = mybir.dt.float8e4
EXP = mybir.ActivationFunctionType.Exp
ALU = mybir.AluOpType
AXX = mybir.AxisListType.X
DR = mybir.MatmulPerfMode.DoubleRow

# fp8 cross-term scales (product of each pair = 1.0)
S_LX = 512.0      # lo(X) * 2^9   (stationary, term1)
S_HA = 1.0 / 512.0  # hi(A) * 2^-9  (moving, term1)
S_HX = 1.0 / 16.0   # hi(X) * 2^-4  (stationary, term2)
S_LA = 16.0         # lo(A) * 2^4   (moving, term2)


def _chunks(total, step=512):
    return [(lo, min(lo + step, total)) for lo in range(0, total, step)]


def build_core_kernel(S, D, B, sbn=SBN, aug=AUG):
    """One core's kernel: query rows block of size B, full S keys."""
    KT = D // P      # contraction tiles over D
    NT = S // P      # key tiles
    MT = B // P      # query tiles (per core)
    NSB = NT // sbn  # super-blocks
    NPAIR = KT // 2  # DoubleRow contraction pairs
    XAW = D + aug
    assert NT % sbn == 0 and B % P == 0 and D % P == 0 and MT <= P and KT % 2 == 0

    nc = bacc.Bacc("TRN2", target_bir_lowering=False, debug=False)
    # All hi/lo f32r splits are precomputed on the host: hi values carry an
    # 11-bit mantissa (so the on-device f32r cast that satisfies the BIR
    # verifier is numerically a no-op), lo = x - hi exactly. The fp8 cross
    # operands for X ship pre-quantized in the natural (plain DoubleRow)
    # stationary layout.
    xtj = nc.dram_tensor("xtj", [NT, P, D], F32, kind="ExternalInput")
    xl8 = nc.dram_tensor("xl8", [NT, P, D], F8, kind="ExternalInput")
    xh8 = nc.dram_tensor("xh8", [NT, P, D], F8, kind="ExternalInput")
    xa = nc.dram_tensor("xa", [S, XAW], BF16, kind="ExternalInput")
    wqh = nc.dram_tensor("wqh", [KT, P, D], F32, kind="ExternalInput")
    wql = nc.dram_tensor("wql", [KT, P, D], F32, kind="ExternalInput")
    wkh = nc.dram_tensor("wkh", [KT, P, D], F32, kind="ExternalInput")
    wkl = nc.dram_tensor("wkl", [KT, P, D], F32, kind="ExternalInput")
    xih = nc.dram_tensor("xih", [KT, P, B], F32, kind="ExternalInput")
    xil = nc.dram_tensor("xil", [KT, P, B], F32, kind="ExternalInput")
    out = nc.dram_tensor("out", [B, D], F32, kind="ExternalOutput")
    dscr = nc.dram_tensor("dscr", [1, B], F32, kind="Internal")

    def pair_st(t, u):
        # stationary fp8 pair view [P, 2, P] of a [P, D] tile, pair u
        return t[:, u * 2 * P:(u + 1) * 2 * P].rearrange("p (i m) -> p i m", i=2)

    with tile.TileContext(nc) as tc, ExitStack() as ctx:
        pers = ctx.enter_context(tc.tile_pool(name="pers", bufs=1))
        aith = [pers.tile([P, B], F32R, name=f"aith{k}") for k in range(KT)]
        h8a = pers.tile([P, KT, B], F8, name="h8a")
        l8a = pers.tile([P, KT, B], F8, name="l8a")
        gm = pers.tile([P, B], F32, name="gm")
        mxbc = pers.tile([P, B], F32, name="mxbc")
        ident = pers.tile([P, P], F32, name="ident")
        ones = pers.tile([P, P], BF16, name="ones")
        den = pers.tile([1, B], F32, name="den")
        make_identity(nc, ident[:])
        nc.gpsimd.memset(gm[:], -1e30)
        nc.gpsimd.memset(ones[:], 1.0)
        nc.gpsimd.memset(den[:], 0.0)

        # ---- phase 0: T1 = Wqs^T X_i^T ; A^T = Wk T1 (per B-half) ----
        with ExitStack() as p0:
            t1p = p0.enter_context(tc.tile_pool(name="t1p", bufs=1))
            wp = p0.enter_context(tc.tile_pool(name="wp", bufs=2))
            xip = p0.enter_context(tc.tile_pool(name="xip", bufs=1))
            auxp = p0.enter_context(tc.tile_pool(name="auxp", bufs=2))
            ps0 = p0.enter_context(tc.tile_pool(name="ps0", bufs=4, space="PSUM"))
            HB = 512
            hT1 = t1p.tile([P, KT, HB], F32R, name="hT1")
            lT1 = t1p.tile([P, KT, HB], F32R, name="lT1")
            def load_w(dh, dl, col, tag):
                # one strided DMA per hi/lo + f32r casts (the rounding casts
                # keep the BIR verifier happy; numerically no-ops)
                wf_h = wp.tile([P, KT, P], F32, name=f"wfh{tag}", tag="wqfh")
                wf_l = wp.tile([P, KT, P], F32, name=f"wfl{tag}", tag="wqfl")
                nc.sync.dma_start(wf_h[:], dh.ap()[:, :, col * P:(col + 1) * P].rearrange("g p c -> p g c"))
                nc.sync.dma_start(wf_l[:], dl.ap()[:, :, col * P:(col + 1) * P].rearrange("g p c -> p g c"))
                hw = wp.tile([P, KT, P], F32R, name=f"hw{tag}", tag="hwq")
                nc.scalar.copy(hw[:], wf_h[:])
                lw = wp.tile([P, KT, P], F32R, name=f"lw{tag}", tag="lwq")
                nc.vector.tensor_copy(lw[:], wf_l[:])
                return hw, lw

            for (lo, hi) in _chunks(B):
                # wq r=0 first: its DMA would otherwise queue behind all 16
                # X-split DMAs and delay the first matmul by ~10us
                wq0 = load_w(wqh, wql, 0, f"q{lo}_0")

                # X_i^T half: host-split hi/lo, device just casts to f32r
                # (numeric no-op; satisfies the verifier's rounding rule),
                # pipelined per g-tile so step1 starts as soon as g=0 lands
                xif_h = xip.tile([P, KT, HB], F32, name=f"xifh{lo}", tag="xifh")
                xif_l = xip.tile([P, KT, HB], F32, name=f"xifl{lo}", tag="xifl")
                hxi = xip.tile([P, KT, HB], F32R, name=f"hxi{lo}", tag="hxi")
                lxi = xip.tile([P, KT, HB], F32R, name=f"lxi{lo}", tag="lxi")
                for g in range(KT):
                    nc.sync.dma_start(xif_h[:, g, :], xih.ap()[g, :, lo:hi])
                    nc.scalar.copy(hxi[:, g, :], xif_h[:, g, :])
                    nc.sync.dma_start(xif_l[:, g, :], xil.ap()[g, :, lo:hi])
                    nc.vector.tensor_copy(lxi[:, g, :], xif_l[:, g, :])

                # step1: T1[r-tile, half] = sum_g Wqs[g,:][:, r]^T X^T[g, half]
                for r in range(KT):
                    hwq, lwq = wq0 if r == 0 else load_w(wqh, wql, r, f"q{lo}_{r}")
                    t1ps = ps0.tile([P, HB], F32, name=f"t1ps{lo}_{r}", tag="pm")
                    n = 3 * KT
                    i = 0
                    for g in range(KT):
                        for (la, rb) in ((hwq, hxi), (hwq, lxi), (lwq, hxi)):
                            nc.tensor.matmul(t1ps[:], la[:, g, :], rb[:, g, :],
                                             start=(i == 0), stop=(i == n - 1))
                            i += 1
                    nc.vector.tensor_copy(hT1[:, r, :], t1ps[:])
                    nc.vector.tensor_sub(t1ps[:], t1ps[:], hT1[:, r, :].bitcast(F32))
                    nc.vector.tensor_copy(lT1[:, r, :], t1ps[:])

                # step2: A^T[d-tile, half] = sum_r Wk[:, r][d, :] T1[r, half]
                # (reuses the step1 weight-split slots: same tags/shapes)
                for d in range(KT):
                    hwk, lwk = load_w(wkh, wkl, d, f"k{lo}_{d}")
                    pa = ps0.tile([P, HB], F32, name=f"pa{lo}_{d}", tag="pm")
                    n = 3 * KT
                    i = 0
                    for r in range(KT):
                        for (la, rb) in ((hwk, hT1), (hwk, lT1), (lwk, hT1)):
                            nc.tensor.matmul(pa[:], la[:, r, :], rb[:, r, :],
                                             start=(i == 0), stop=(i == n - 1))
                            i += 1
                    nc.vector.tensor_copy(aith[d][:, lo:hi], pa[:])
                    nc.scalar.mul(h8a[:, d, lo:hi], aith[d][:, lo:hi].bitcast(F32), S_HA)
                    al_f = auxp.tile([P, HB], F32, name=f"alf{lo}_{d}", tag="alf")
                    nc.vector.tensor_sub(al_f[:], pa[:], aith[d][:, lo:hi].bitcast(F32))
                    nc.vector.tensor_scalar_mul(l8a[:, d, lo:hi], al_f[:], S_LA)

        # ---- flash over key super-blocks ----
        accp = ctx.enter_context(tc.tile_pool(name="accp", bufs=1))
        acc = [accp.tile([P, XAW], F32, name=f"acc{t}") for t in range(MT)]
        for t in range(MT):
            nc.gpsimd.memset(acc[t][:], 0.0)
        sp = ctx.enter_context(tc.tile_pool(name="sp", bufs=2 * sbn + 2))
        erp = ctx.enter_context(tc.tile_pool(name="erp", bufs=2 * sbn))
        xap = ctx.enter_context(tc.tile_pool(name="xap", bufs=3))
        xarp = ctx.enter_context(tc.tile_pool(name="xarp", bufs=2 * sbn))
        xtp = xap
        xthp = ctx.enter_context(tc.tile_pool(name="xthp", bufs=3))
        stat = ctx.enter_context(tc.tile_pool(name="stat", bufs=2))
        ps_s = ctx.enter_context(tc.tile_pool(name="ps_s", bufs=2, space="PSUM"))
        ps_o = ctx.enter_context(tc.tile_pool(name="ps_o", bufs=2, space="PSUM"))
        ps_d = ctx.enter_context(tc.tile_pool(name="ps_d", bufs=1, space="PSUM"))
        ps_t = ps_s

        # Software pipeline, one-super-block lag, tuned so the PE queue is
        # [S(s) | out(s-1) | transposes(s) | S(s+1) | ...] with no waits:
        # out(s-1)'s operands (er/xar/corr) were finished during S(s)'s burst,
        # and the gm column-maxes feeding transposes(s) finish during
        # out(s-1). The exp chain for s runs on DVE/ScalarE under S(s+1).
        # E and X_aug are bf16 for the out matmul (same 1 cyc/row as f32r,
        # half the SBUF; E's 2^-9 rounding cancels through the ones-column
        # denominator, X_aug's is ~2e-3 of |x| — both far under budget).
        # X_aug ships from the host already in bf16, so no on-device cast.
        def prep_block(s):
            js = list(range(s * sbn, (s + 1) * sbn))
            xsplit = []
            for j in js:
                xt_t = xtp.tile([P, D], F32, name=f"xt{j}", tag="stg")
                nc.sync.dma_start(xt_t[:], xtj.ap()[j])
                xth = xthp.tile([P, D], F32R, name=f"xth{j}", tag="xth")
                nc.scalar.copy(xth[:], xt_t[:])
                l8x = xthp.tile([P, D], F8, name=f"l8x{j}", tag="l8x")
                nc.sync.dma_start(l8x[:], xl8.ap()[j])
                h8x = xthp.tile([P, D], F8, name=f"h8x{j}", tag="h8x")
                nc.sync.dma_start(h8x[:], xh8.ap()[j])
                xsplit.append((xth, l8x, h8x))
            return xsplit

        def s_burst(s, xsplit):
            ssb = []
            for idx, j in enumerate(range(s * sbn, (s + 1) * sbn)):
                xth, l8x, h8x = xsplit[idx]
                s_t = sp.tile([P, B], F32, name=f"s{j}", tag="s")
                pss = [ps_s.tile([P, 512], F32, name=f"pss{j}_{c}", tag="pss")
                       for c in range(2)]
                # f32r hh pass, both chunks back-to-back (same PE mode)
                for c, (lo, hi) in enumerate(_chunks(B)):
                    for k in range(KT):
                        nc.tensor.matmul(pss[c][:], xth[:, k * P:(k + 1) * P],
                                         aith[k][:, lo:hi], start=(k == 0), stop=(k == KT - 1))
                # fp8 DoubleRow cross passes: each accumulates onto the
                # closed f32r group via has_written (start=False); stop is
                # sim-only bookkeeping so every DR matmul closes itself.
                # Chunk 0 drains (copy + running max) while chunk 1's fp8
                # matmuls stream, so the stats transposes can start the
                # moment the burst ends.
                for c, (lo, hi) in enumerate(_chunks(B)):
                    for u in range(NPAIR):
                        nc.tensor.matmul(pss[c][:], pair_st(l8x, u),
                                         h8a[:, 2 * u:2 * u + 2, lo:hi],
                                         start=False, stop=True, perf_mode=DR,
                                         skip_group_check=True)
                    for u in range(NPAIR):
                        nc.tensor.matmul(pss[c][:], pair_st(h8x, u),
                                         l8a[:, 2 * u:2 * u + 2, lo:hi],
                                         start=False, stop=True, perf_mode=DR,
                                         skip_group_check=True)
                    nc.scalar.copy(s_t[:, lo:hi], pss[c][:])
                    nc.vector.tensor_max(gm[:, lo:hi], gm[:, lo:hi], pss[c][:])
                ssb.append(s_t)
            return ssb

        def stats_block(s, omx):
            # per-query-column running max (transpose-reduce gm chunks)
            nmx = stat.tile([P, MT], F32, name=f"nmx{s}", tag="nmx")
            corr = stat.tile([P, MT], F32, name=f"corr{s}", tag="corr")
            for c in range(MT):
                pt = ps_t.tile([P, P], F32, name=f"pt{s}_{c}", tag="pss")
                nc.tensor.transpose(pt[:], gm[:, c * P:(c + 1) * P], ident[:])
                nc.vector.reduce_max(nmx[:, c:c + 1], pt[:], axis=AXX)
            if omx is None:
                nc.vector.memset(corr[:], 0.0)
            else:
                dmx = stat.tile([P, MT], F32, name=f"dmx{s}", tag="dmx")
                nc.vector.tensor_sub(dmx[:], omx[:], nmx[:])
                nc.scalar.activation(corr[:], dmx[:], EXP)

            # broadcast nmx (query-major) -> mxbc [P, B] (key-major free)
            ptb = ps_t.tile([P, P], F32, name=f"ptb{s}", tag="pss")
            nc.tensor.transpose(ptb[:MT, :], nmx[:], ident[:])
            mtmp = stat.tile([MT, P], F32, name=f"mtmp{s}", tag="mtmp")
            nc.scalar.copy(mtmp[:], ptb[:MT, :])
            # issue the tiny mrow DMA from the scalar queue so it doesn't
            # sit behind the bulk xtj/xa loads on the sync queue
            mrow = stat.tile([1, B], F32, name=f"mrow{s}", tag="mrow", bufs=1)
            nc.scalar.dma_start(mrow[:].rearrange("a (b c) -> a b c", b=MT), mtmp[:])
            nc.gpsimd.partition_broadcast(mxbc[:], mrow[:])
            return nmx, corr

        def exp_block(s, ssb):
            # E = exp(S - max), exp writes bf16 er (out dtype converts).
            # Chunked [P, 512] with chunk 0 of every tile first: the out
            # burst consumes er columns t*128.. in t order, so all its
            # early stationaries come from chunk 0 — this halves the time
            # from max-broadcast to out-burst start.
            ers = [erp.tile([P, B], BF16, name=f"er{s}_{idx}", tag="er")
                   for idx in range(len(ssb))]
            for (lo, hi) in _chunks(B):
                for idx, s_t in enumerate(ssb):
                    nc.vector.tensor_sub(s_t[:, lo:hi], s_t[:, lo:hi], mxbc[:, lo:hi])
                    nc.scalar.activation(ers[idx][:, lo:hi], s_t[:, lo:hi], EXP)
            return ers

        def xar_block(s):
            xar = []
            for j in range(s * sbn, (s + 1) * sbn):
                xa_t = xarp.tile([P, XAW], BF16, name=f"xa{j}", tag="xar")
                nc.sync.dma_start(xa_t[:], xa.ap()[j * P:(j + 1) * P, :])
                xar.append(xa_t)
            return xar

        def den_update(s, ers, corr):
            # softmax denominator via ones-stationary matmuls (row layout),
            # rescaled with crow = corr transposed to row-major (the same
            # transpose/flatten-DMA idiom as the mrow broadcast)
            dps = ps_d.tile([P, B], F32, name=f"dps{s}", tag="dps")
            for (lo, hi) in _chunks(B):
                for idx in range(sbn):
                    nc.tensor.matmul(dps[:, lo:hi], ones[:], ers[idx][:, lo:hi],
                                     start=(idx == 0), stop=(idx == sbn - 1))
            ptc = ps_t.tile([P, P], F32, name=f"ptc{s}", tag="pss")
            nc.tensor.transpose(ptc[:MT, :], corr[:], ident[:])
            ctmp = stat.tile([MT, P], F32, name=f"ctmp{s}", tag="mtmp")
            nc.scalar.copy(ctmp[:], ptc[:MT, :])
            crow = stat.tile([1, B], F32, name=f"crow{s}", tag="crow", bufs=1)
            nc.scalar.dma_start(crow[:].rearrange("a (b c) -> a b c", b=MT), ctmp[:])
            nc.vector.tensor_mul(den[:], den[:], crow[:])
            nc.vector.tensor_add(den[:], den[:], dps[0:1, :])

        def out_block(s, ers, xar, corr, final=False):
            # out accumulation: acc = acc*corr + E^T @ X (bf16 burst).
            # On the final block the denominator runs FIRST so its
            # row->query-major DRAM bounce finishes during the burst and the
            # per-tile divides+stores pipeline with the matmuls.
            rcd = None
            if final:
                den_update(s, ers, corr)
                nc.sync.dma_start(dscr.ap()[:, :], den[:])
                dtmp = stat.tile([P, P], F32, name="dtmp", tag="dtmp")
                nc.gpsimd.memset(dtmp[:], 0.0)
                nc.sync.dma_start(dtmp[:MT, :], dscr.ap()[0, :].rearrange("(b c) -> b c", b=MT))
            for t in range(MT):
                po = ps_o.tile([P, XAW], F32, name=f"po{s}_{t}", tag="po")
                # idx outer so the column chunks reuse one stationary
                # operand back-to-back (LDWEIGHTS locality); each chunk's
                # PSUM accumulation group still spans idx 0..sbn-1
                for idx in range(sbn):
                    er = ers[idx][:]
                    for (lo, hi) in _chunks(XAW):
                        nc.tensor.matmul(po[:, lo:hi], er[:, t * P:(t + 1) * P],
                                         xar[idx][:, lo:hi], start=(idx == 0), stop=(idx == sbn - 1))
                nc.vector.scalar_tensor_tensor(acc[t][:], acc[t][:],
                                               corr[:, t:t + 1], po[:],
                                               op0=ALU.mult, op1=ALU.add)
                if final and t == 2:
                    ptd = ps_t.tile([P, P], F32, name="ptd", tag="pss")
                    nc.tensor.transpose(ptd[:], dtmp[:], ident[:])
                    rcd = stat.tile([P, MT], F32, name="rcd", tag="rcd")
                    nc.vector.reciprocal(rcd[:], ptd[:, :MT])
                if final and t >= 2:
                    for tt in ([0, 1, 2] if t == 2 else [t]):
                        nc.vector.tensor_scalar_mul(acc[tt][:], acc[tt][:], rcd[:, tt:tt + 1])
                        eng = nc.sync if tt % 2 == 0 else nc.scalar
                        eng.dma_start(out.ap()[tt * P:(tt + 1) * P, :], acc[tt][:])
            if not final:
                den_update(s, ers, corr)

        omx = None
        prev = None     # out_block args for block s-1
        xsplit = prep_block(0)
        for s in range(NSB):
            ssb = s_burst(s, xsplit)
            if s + 1 < NSB:
                xsplit = prep_block(s + 1)
            # stats(s) on PE right after the burst (its gm maxes already
            # drained), so the max-broadcast/exp chain overlaps out(s-1)
            nmx, corr = stats_block(s, omx)
            omx = nmx
            if prev is not None:
                out_block(s - 1, *prev)
            ers = exp_block(s, ssb)
            xar = xar_block(s)
            prev = (ers, xar, corr)
        out_block(NSB - 1, *prev, final=True)

    nc.compile()
    return nc


def _split_f32r(x):
    """Host replica of the f32r hi/lo split: hi = x rounded (half-up) to an
    11-bit mantissa — so the PE's FP22 truncation and the device's f32r cast
    both read it back exactly — and lo = x - hi, exact in fp32."""
    x = np.ascontiguousarray(x, np.float32)
    hi = ((x.view(np.uint32) + np.uint32(0x800)) & np.uint32(0xFFFFF000)).view(np.float32)
    return hi, (x - hi).astype(np.float32)


def prep_inputs(X, Wq, Wk, S, D, n_cores, aug=AUG):
    import ml_dtypes
    F8NP = ml_dtypes.float8_e4m3
    B = S // n_cores
    NT = S // P
    KT = D // P
    X = np.ascontiguousarray(X, np.float32)
    scale = np.float32(1.0 / np.sqrt(D))
    xtj = np.ascontiguousarray(
        X.reshape(NT, P, KT, P).transpose(0, 3, 2, 1).reshape(NT, P, D))
    xtj_hi, xtj_lo = _split_f32r(xtj)
    xl8 = (xtj_lo * np.float32(S_LX)).astype(F8NP)
    xh8 = (xtj_hi * np.float32(S_HX)).astype(F8NP)
    xa = np.zeros((S, D + aug), ml_dtypes.bfloat16)
    xa[:, :D] = X.astype(ml_dtypes.bfloat16)
    if aug:
        xa[:, D] = 1.0
    wqh, wql = _split_f32r(np.asarray(Wq, np.float32) * scale)
    wkh, wkl = _split_f32r(np.asarray(Wk, np.float32).T)
    xih_full, xil_full = _split_f32r(X.T)
    in_maps = []
    for i in range(n_cores):
        in_maps.append({
            "xtj": xtj_hi, "xl8": xl8, "xh8": xh8, "xa": xa,
            "wqh": wqh.reshape(KT, P, D), "wql": wql.reshape(KT, P, D),
            "wkh": wkh.reshape(KT, P, D), "wkl": wkl.reshape(KT, P, D),
            "xih": np.ascontiguousarray(xih_full[:, i * B:(i + 1) * B]).reshape(KT, P, B),
            "xil": np.ascontiguousarray(xil_full[:, i * B:(i + 1) * B]).reshape(KT, P, B),
        })
    return in_maps


_CACHE = {}


def _get_kernel(S, D, B, sbn):
    key = (S, D, B, sbn)
    if key not in _CACHE:
        _CACHE[key] = build_core_kernel(S, D, B, sbn=sbn)
    return _CACHE[key]


def kernel(inputs, weight_query, weight_key):
    S, D = inputs.shape
    assert (S, D) == (SEQ, DIM)
    B = S // NCORES
    nc = _get_kernel(S, D, B, SBN)
    in_maps = prep_inputs(inputs, weight_query, weight_key, S, D, NCORES)
    res = run_bass_kernel_spmd(nc, in_maps, core_ids=list(range(NCORES)))
    return np.concatenate([res.results[i]["out"] for i in range(NCORES)], axis=0)


if __name__ == "__main__":
    rng = np.random.default_rng(0)
    X = rng.standard_normal((SEQ, DIM), dtype=np.float32)
    Wq = rng.standard_normal((DIM, DIM), dtype=np.float32)
    Wk = rng.standard_normal((DIM, DIM), dtype=np.float32)
    out = kernel(X, Wq, Wk)
    print(out.shape, out.dtype)
